# revision 1
# baseline (speedup 1.0000x reference)
"""Multi-head attention (RoPE, causal, fp32) on 8 Trainium2 NeuronCores.

Problem: B=2, S=2048, D=2048, H=16 heads (hd=128).
Sharding: DP=2 (batch) x TP=4 (head groups of 4 heads). Core c handles
batch c//4, head group c%4. Each core computes q/k/v projections for its
512 features, RoPE, causal attention, and a partial o_proj against its
512 columns of Wo. The host sums the 4 partial o_proj outputs per batch.

Kernel layout strategy (per core):
  - qT, kT in [hd, seq] ("transposed") layout straight out of the
    projection matmuls; v in natural [seq, feat] layout. RoPE applied in
    place at eviction time (rowswap via SBUF->SBUF DMA, sign baked into
    the host-provided sin table).
  - Attention entirely in transposed space: scoresT[k, q] tiles from
    lhsT=kT slice, rhs=qT chunk, N=512. exp fused into the PSUM
    eviction on ScalarE (scale=1/sqrt(hd)), software-pipelined with the
    denominator / attn@V accumulation matmuls two steps behind.
    Causal: only j <= q k-tiles are computed; on the diagonal tile the
    fully-masked 128-wide sub-blocks are zeroed and a single shared
    [128,128] triangular mask is multiplied in.
    Softmax denominator via an all-ones [128,128] stationary matmul
    (yields the k-sum pre-broadcast across partitions); 1/denom via
    4 split VectorE reciprocals; normalization folded into the attn@V
    PSUM eviction.
  - o_proj is weight-stationary and emits the partial TRANSPOSED
    ([D_out, S]); the host transposes back while summing the 4 per-batch
    partials.
All matmuls run as float32r (fp32_mode=HIGH single-pass: full column
rate on fp32 data, ~1e-4-grade precision). Producers of f32r-consumed
SBUF data must write f32r-rounded outputs (BIR verifier requirement).
"""

import sys

for _p in ("/opt/trn_rl_repo",):
    if _p not in sys.path:
        sys.path.insert(0, _p)

import numpy as np

import concourse.bass as bass
import concourse.mybir as mybir
import concourse.tile as tile
from concourse import bacc, bass_utils


def _enable_ldw_opt():
    """walrus ships with --enable-ldw-opt=false; turning it on lets codegen
    elide weight reloads for consecutive matmuls sharing a stationary
    operand (the o_proj and denominator matmuls rely on this)."""
    if getattr(bass_utils, "_ldw_opt_patched", False):
        return
    orig = bass_utils.run_command

    def patched(argv, **kw):
        argv = ["--enable-ldw-opt=true" if a == "--enable-ldw-opt=false" else a
                for a in argv]
        return orig(argv, **kw)

    bass_utils.run_command = patched
    bass_utils._ldw_opt_patched = True


_enable_ldw_opt()

P = 128          # partitions / head dim
S = 2048         # sequence length
D = 2048         # model dim
F = 512          # features per core (4 heads)
H = 4            # heads per core
HD = 128         # head dim
NJ = D // P      # 16 contraction chunks of 128
NQ = S // 512    # 4 query chunks of 512
SCALE = 1.0 / float(np.sqrt(HD))

F32 = mybir.dt.float32
F32R = mybir.dt.float32r
AFT = mybir.ActivationFunctionType


def _r(ap):
    """View an fp32 AP as float32r for full-rate PE matmuls."""
    return ap.bitcast(F32R)


def _body(tc, xT, wqT, wkT, wvT, woT, cosT, sinT, mskT, out):
    nc = tc.nc

    # long-lived slabs with hand-managed lifetimes; pools are per-side LIFO
    # stacks, so the q/k/v slabs live on the "left" stack while phase-local
    # pools and the oT slab (which outlives q/k/v) use the default side.
    p_qk = tc.alloc_tile_pool(name="p_qk", bufs=1, side="left")   # phases 1..3
    qT = p_qk.tile([P, H, S], F32)    # [hd, head, seq]
    kT = p_qk.tile([P, H, S], F32)

    p_v = tc.alloc_tile_pool(name="p_v", bufs=1, side="left")     # phases 1..3
    vN = p_v.tile([P, NJ, F], F32)   # [:, j, :] = v[j*128:(j+1)*128, :]

    # ---------------- projections: q, k (transposed layout) + RoPE ----
    p_xs = tc.alloc_tile_pool(name="p_xs", bufs=6, side="left")   # phases 1..2
    with tc.tile_pool(name="cs", bufs=1) as cspool, \
         tc.tile_pool(name="wqk", bufs=1) as wpool, \
         tc.tile_pool(name="rot", bufs=4) as rpool, \
         tc.tile_pool(name="pp", bufs=1, space="PSUM") as pp:
        cos_sb = cspool.tile([P, S], F32)
        sin_sb = cspool.tile([P, S], F32)
        wq_sb = wpool.tile([P, NJ, F], F32R)
        wk_sb = wpool.tile([P, NJ, F], F32R)
        for s in range(NQ):
            pq = [pp.tile([P, 512], F32, name=f"pq{s}_{h}", tag=f"pq{h}")
                  for h in range(H)]
            pk = [pp.tile([P, 512], F32, name=f"pk{s}_{h}", tag=f"pk{h}")
                  for h in range(H)]
            for j in range(NJ):
                if s == 0:
                    # weight loads interleaved with the first x chunks so
                    # the first matmuls aren't stuck behind 8MB of DMA
                    nc.sync.dma_start(wq_sb[:, j, :], wqT[j * P:(j + 1) * P, :])
                    nc.sync.dma_start(wk_sb[:, j, :], wkT[j * P:(j + 1) * P, :])
                xt = p_xs.tile([P, 512], F32R, name=f"xt{s}_{j}", tag="xt")
                nc.sync.dma_start(xt[:], xT[j * P:(j + 1) * P, s * 512:(s + 1) * 512])
                if s == 0 and j == 0:
                    nc.sync.dma_start(cos_sb[:], cosT)
                    nc.sync.dma_start(sin_sb[:], sinT)
                for h in range(H):
                    nc.tensor.matmul(pq[h][:], _r(wq_sb[:, j, h * HD:(h + 1) * HD]),
                                     _r(xt[:]), start=(j == 0), stop=(j == NJ - 1))
                    nc.tensor.matmul(pk[h][:], _r(wk_sb[:, j, h * HD:(h + 1) * HD]),
                                     _r(xt[:]), start=(j == 0), stop=(j == NJ - 1))
            sl = slice(s * 512, (s + 1) * 512)
            # evictions first (split ScalarE/VectorE) so all 8 PSUM banks
            # free quickly for the next seq chunk's matmuls...
            dsts = []
            for h in range(H):
                for ti, (ps, slab) in enumerate(((pq[h], qT), (pk[h], kT))):
                    dst = slab[:, h, sl]
                    if ti == 0:
                        nc.scalar.activation(dst.bitcast(F32R), ps[:], AFT.Copy)
                    else:
                        nc.vector.tensor_copy(dst.bitcast(F32R), ps[:])
                    dsts.append((s, h, dst))
            # ...then RoPE in place: dst = dst*cos + rowswap(dst)*sin
            # (all on VectorE: GpSimd shares the DVE SBUF port, so moving
            # work there slows both engines down)
            for sh, h, dst in dsts:
                rt = rpool.tile([P, 512], F32, name=f"rt{sh}_{h}", tag="rt")
                nc.sync.dma_start(rt[0:64, :], dst[64:128, :])
                nc.sync.dma_start(rt[64:128, :], dst[0:64, :])
                nc.vector.tensor_mul(rt[:], rt[:], sin_sb[:, sl])
                nc.vector.tensor_mul(dst.bitcast(F32R), dst, cos_sb[:, sl])
                nc.vector.tensor_add(dst.bitcast(F32R), dst, rt[:])

    # ---------------- projection: v (natural layout) ------------------
    # wv lives on the RIGHT stack: fresh space, so its loads don't
    # WAR-wait on the phase-1 RoPE tail that reads cos/sin/rot
    p_wv = tc.alloc_tile_pool(name="p_wv", bufs=1, side="right")
    wv_sb = p_wv.tile([P, NJ, F], F32R)
    with tc.tile_pool(name="pv", bufs=1, space="PSUM") as pvp:
        for sg in range(4):
            pv = [pvp.tile([P, F], F32, name=f"pv{sg}_{st}", tag=f"pv{st}")
                  for st in range(4)]
            for j in range(NJ):
                if sg == 0:
                    nc.sync.dma_start(wv_sb[:, j, :], wvT[j * P:(j + 1) * P, :])
                xt2 = p_xs.tile([P, 512], F32R, name=f"x2{sg}_{j}", tag="xt")
                nc.sync.dma_start(xt2[:], xT[j * P:(j + 1) * P, sg * 512:(sg + 1) * 512])
                for st in range(4):
                    nc.tensor.matmul(pv[st][:], _r(xt2[:, st * P:(st + 1) * P]),
                                     _r(wv_sb[:, j, :]), start=(j == 0), stop=(j == NJ - 1))
            for st in range(4):
                nc.scalar.activation(vN[:, sg * 4 + st, :].bitcast(F32R), pv[st][:], AFT.Copy)

    p_wv.release()
    p_xs.release()

    # ---------------- attention (all in transposed space) -------------
    # attention-phase SBUF lives on the RIGHT stack so nothing here reuses
    # the just-released wv/xs2 space (which would add WAR waits on the tail
    # of the v pass). Wo is prefetched here too, for the same reason plus
    # DMA overlap with attention compute.
    p_oT = tc.alloc_tile_pool(name="p_oT", bufs=1, side="right")  # phases 3..4
    oT = p_oT.tile([P, H, S], F32)    # attention output, transposed
    with tc.tile_pool(name="amsk", bufs=1, side="right") as mpool, \
         tc.tile_pool(name="exp", bufs=1, side="right") as epool, \
         tc.tile_pool(name="attsb", bufs=2, side="right") as apool, \
         tc.tile_pool(name="pa", bufs=1, space="PSUM") as pap:
        msk_sb = mpool.tile([P, P], F32)
        nc.sync.dma_start(msk_sb[:], mskT)
        # all-ones [128,128] stationary: the denominator matmul then yields
        # the k-sum already broadcast across all 128 partitions of PSUM.
        ones_tmp = mpool.tile([P, P], F32)
        nc.vector.memset(ones_tmp[:], 1.0)
        ones_mat = mpool.tile([P, P], F32)
        nc.vector.tensor_copy(ones_mat[:].bitcast(F32R), ones_tmp[:])
        for h in range(H):
            qh = qT[:, h, :]
            kh = kT[:, h, :]
            for q in range(NQ):
                jmax = 4 * (q + 1)
                qsl = slice(q * 512, (q + 1) * 512)
                ex = epool.tile([P, NJ, 512], F32, name=f"ex{h}_{q}", tag="ex")
                pden = pap.tile([P, 512], F32, name=f"pden{h}{q}", tag="pden", bufs=2)
                pov = pap.tile([P, 512], F32, name=f"pov{h}{q}", tag="pov", bufs=2)

                def _mask(j):
                    if j >= 4 * q:
                        d = j - 4 * q
                        if d > 0:
                            # x*0 write: memset can't emit an f32r-rounded
                            # store, a zero-scaled tensor_scalar can
                            nc.vector.tensor_scalar_mul(
                                ex[:, j, 0:d * P].bitcast(F32R), ex[:, j, 0:d * P], 0.0)
                        nc.vector.tensor_mul(
                            ex[:, j, d * P:(d + 1) * P].bitcast(F32R),
                            ex[:, j, d * P:(d + 1) * P], msk_sb[:])

                def score_step(j):
                    # pairs of score tiles share a 2-bank PSUM tile so one
                    # ScalarE exp op evicts both (halves ACT op overhead,
                    # the attention pace-setter)
                    psc = pap.tile([P, 2, 512], F32, name=f"psc{h}{q}{j}",
                                   tag="psc", bufs=2)
                    for t in range(2):
                        nc.tensor.matmul(psc[:, t, :], _r(kh[:, (j + t) * P:(j + t + 1) * P]),
                                         _r(qh[:, qsl]), start=True, stop=True)
                    nc.scalar.activation(ex[:, j:j + 2, :].bitcast(F32R), psc[:],
                                         AFT.Exp, scale=SCALE)
                    _mask(j)
                    _mask(j + 1)

                def acc_step(j):
                    nc.tensor.matmul(pden[:], _r(ones_mat[:]), _r(ex[:, j, :]),
                                     start=(j == 0), stop=(j == jmax - 1))
                    nc.tensor.matmul(pov[:], _r(vN[:, j, h * HD:(h + 1) * HD]),
                                     _r(ex[:, j, :]), start=(j == 0), stop=(j == jmax - 1))

                # software-pipelined: score pairs run one pair ahead of
                # the denominator/attn@V accumulation consuming exp
                for j in range(0, jmax, 2):
                    score_step(j)
                    if j >= 2:
                        acc_step(j - 2)
                        acc_step(j - 1)
                acc_step(jmax - 2)
                acc_step(jmax - 1)
                rbc = apool.tile([P, 512], F32, name=f"rbc{h}{q}", tag="rbc")
                for rc in range(4):
                    nc.vector.reciprocal(rbc[:, rc * P:(rc + 1) * P],
                                         pden[:, rc * P:(rc + 1) * P])
                nc.vector.tensor_mul(oT[:, h, qsl].bitcast(F32R), pov[:], rbc[:])

    p_v.release()
    p_qk.release()

    # ---------------- o_proj (partial against this core's Wo cols) ----
    # weight-stationary: lhsT = Wo chunk reused across all 4 q-chunks.
    # Output is produced TRANSPOSED ([D_out, S]); the host transposes back.
    with tc.tile_pool(name="wo", bufs=1) as wopool, \
         tc.tile_pool(name="oev", bufs=4) as oevp, \
         tc.tile_pool(name="po", bufs=1, space="PSUM") as pop:
        wo_sb = wopool.tile([P, H, D], F32R)
        for h in range(H):
            nc.sync.dma_start(wo_sb[:, h, :], woT[h * P:(h + 1) * P, :])
        for dt in range(D // P):
            po = [pop.tile([P, 512], F32, name=f"po{dt}_{qc}", tag=f"po{qc}", bufs=2)
                  for qc in range(NQ)]
            for h in range(H):
                for qc in range(NQ):
                    nc.tensor.matmul(po[qc][:], _r(wo_sb[:, h, dt * P:(dt + 1) * P]),
                                     _r(oT[:, h, qc * 512:(qc + 1) * 512]),
                                     start=(h == 0), stop=(h == H - 1))
            for qc in range(NQ):
                ot = oevp.tile([P, 512], F32, name=f"ot{dt}_{qc}", tag="ot")
                if (dt + qc) % 2 == 0:
                    nc.vector.tensor_copy(ot[:], po[qc][:])
                else:
                    nc.scalar.activation(ot[:], po[qc][:], AFT.Copy)
                nc.sync.dma_start(out[dt * P:(dt + 1) * P, qc * 512:(qc + 1) * 512], ot[:])
    p_oT.release()


def build_nc():
    nc = bacc.Bacc("TRN2", target_bir_lowering=False, debug=False,
                   enable_asserts=True, num_devices=8)
    xT = nc.dram_tensor("xT", [D, S], F32R, kind="ExternalInput").ap()
    wqT = nc.dram_tensor("wqT", [D, F], F32R, kind="ExternalInput").ap()
    wkT = nc.dram_tensor("wkT", [D, F], F32R, kind="ExternalInput").ap()
    wvT = nc.dram_tensor("wvT", [D, F], F32R, kind="ExternalInput").ap()
    woT = nc.dram_tensor("woT", [F, D], F32R, kind="ExternalInput").ap()
    cosT = nc.dram_tensor("cosT", [P, S], F32, kind="ExternalInput").ap()
    sinT = nc.dram_tensor("sinT", [P, S], F32, kind="ExternalInput").ap()
    mskT = nc.dram_tensor("mskT", [P, P], F32, kind="ExternalInput").ap()
    out = nc.dram_tensor("out", [S, D], F32, kind="ExternalOutput").ap()

    with tile.TileContext(nc) as tc:
        _body(tc, xT, wqT, wkT, wvT, woT, cosT, sinT, mskT, out)
    nc.compile()
    return nc


_CACHE = {}


def _get_nc():
    if "nc" not in _CACHE:
        _CACHE["nc"] = build_nc()
    return _CACHE["nc"]


def _rope_tables():
    hd = HD
    inv = 1.0 / (10000.0 ** (np.arange(0, hd, 2, dtype=np.float32) / np.float32(hd)))
    t = np.arange(S, dtype=np.float32)
    freqs = np.outer(t, inv)                      # [S, 64]
    emb = np.concatenate([freqs, freqs], axis=-1)  # [S, 128]
    cosT = np.cos(emb).T.astype(np.float32).copy()
    sinT = np.sin(emb).T.astype(np.float32).copy()
    sinT[0:64, :] *= -1.0  # sign of rotate_half baked into the table
    return np.ascontiguousarray(cosT), np.ascontiguousarray(sinT)


def _diag_masks():
    kp = np.arange(P)[:, None]
    qf = np.arange(P)[None, :]
    return np.ascontiguousarray((kp <= qf).astype(np.float32))


def _in_maps(x, Wq, Wk, Wv, Wo):
    cosT, sinT = _rope_tables()
    msk = _diag_masks()
    maps = []
    for c in range(8):
        b, g = c // 4, c % 4
        fs = slice(g * F, (g + 1) * F)
        maps.append({
            "xT": np.ascontiguousarray(x[b].T),
            "wqT": np.ascontiguousarray(Wq[fs, :].T),
            "wkT": np.ascontiguousarray(Wk[fs, :].T),
            "wvT": np.ascontiguousarray(Wv[fs, :].T),
            "woT": np.ascontiguousarray(Wo[:, fs].T),
            "cosT": cosT,
            "sinT": sinT,
            "mskT": msk,
        })
    return maps


def run(x, Wq, Wk, Wv, Wo, trace=False, **spmd_kwargs):
    """Run on 8 cores; returns (full_output, BassKernelResults)."""
    x = np.asarray(x, np.float32)
    Wq = np.asarray(Wq, np.float32)
    Wk = np.asarray(Wk, np.float32)
    Wv = np.asarray(Wv, np.float32)
    Wo = np.asarray(Wo, np.float32)
    nc = _get_nc()
    maps = _in_maps(x, Wq, Wk, Wv, Wo)
    res = bass_utils.run_bass_kernel_spmd(nc, maps, core_ids=list(range(8)),
                                          trace=trace, **spmd_kwargs)
    outs = [res.results[c]["out"] for c in range(8)]
    full = np.empty((2, S, D), np.float32)
    for b in range(2):
        # each core returns its o_proj partial TRANSPOSED ([D_out, S])
        acc = outs[4 * b] + outs[4 * b + 1] + outs[4 * b + 2] + outs[4 * b + 3]
        full[b] = acc.T
    return full, res


def kernel(x, Wq, Wk, Wv, Wo):
    full, _ = run(x, Wq, Wk, Wv, Wo)
    return full



# revision 3
# speedup vs baseline: 1.4421x; 1.4421x over previous
"""Multi-head attention (RoPE, causal, fp32) on 8 Trainium2 NeuronCores.

Problem: B=2, S=2048, D=2048, H=16 heads (hd=128).
Sharding: DP=2 (batch) x TP=4 (head groups of 4 heads). Core c handles
batch c//4, head group c%4. Each core computes q/k/v projections for its
512 features, RoPE, causal attention, and a partial o_proj against its
512 columns of Wo. The host sums the 4 partial o_proj outputs per batch.

v2 layout strategy (per core):
  - x and Wq/Wk/Wv arrive in bf16 (host-cast): halves input DMA and
    enables FWL fast weight loads. All projection matmuls are bf16
    (fp32 PSUM accumulate); attention and o_proj stay float32r.
  - Single fused pass over x: per 512-seq chunk, q+k matmuls for head
    pair 0/1, then head pair 2/3, then v matmuls — all reusing the same
    x tiles in SBUF (x is read from HBM exactly once). PSUM cycles
    through 4 two-bank slots (pq01/pk01/pq23/pk23/pvA/pvB) so the PE
    never waits on evictions.
  - RoPE applied at eviction (rowswap via SBUF->SBUF DMA, sign baked
    into the host-provided sin table), on VectorE.
  - Attention entirely in transposed space: scoresT[k, q] tiles,
    lhsT=kT slice, rhs=qT chunk. exp fused into PSUM eviction on
    ScalarE. One global software pipeline across ALL (head, q-chunk)
    pairs: score j-pair steps run two steps ahead of the denominator /
    attn@V accumulation, with per-j-pair ex tiles so the pipeline never
    drains at (h,q) boundaries.
  - Causal: only j <= q k-tiles computed; diagonal k-tiles compute only
    the alive (q >= k) column range (partial-N matmuls for score, den
    and attn@V), one shared [128,128] triangular mask multiplied on the
    edge block.
  - Softmax denominator via all-ones [128,128] stationary matmul (k-sum
    pre-broadcast across partitions); 1/denom via one DVE
    reciprocal_approx_fast; normalization folded into attn@V eviction.
  - o_proj weight-stationary, emits the partial TRANSPOSED ([D_out, S])
    in bf16; host sums the 4 per-batch partials in fp32.
"""

import sys

for _p in ("/opt/trn_rl_repo",):
    if _p not in sys.path:
        sys.path.insert(0, _p)

import ml_dtypes
import numpy as np

import concourse.bass as bass
import concourse.mybir as mybir
import concourse.tile as tile
from concourse import bacc, bass_utils


# NOTE: the baseline's --enable-ldw-opt=true patch is incompatible with
# bf16 LDWEIGHTS (walrus rejects FWL loads under ldw-opt), so it is not
# used here. LDWEIGHTS issue is hidden under matmul streaming via the
# dual SBUF read ports, so the elision is not needed.

P = 128          # partitions / head dim
S = 2048         # sequence length
D = 2048         # model dim
F = 512          # features per core (4 heads)
H = 4            # heads per core
HD = 128         # head dim
NJ = D // P      # 16 contraction chunks of 128
NQ = S // 512    # 4 query chunks of 512
SCALE = 1.0 / float(np.sqrt(HD))

F32 = mybir.dt.float32
F32R = mybir.dt.float32r
BF16 = mybir.dt.bfloat16
AFT = mybir.ActivationFunctionType


def _r(ap):
    """View an fp32 AP as float32r for full-rate PE matmuls."""
    return ap.bitcast(F32R)


def _proj_phase(tc, xT, wqT, wkT, wvT, cosT, sinT, qT, kT, vN):
    """Fused q/k/v projections + RoPE: one pass over x."""
    nc = tc.nc
    with tc.tile_pool(name="cs", bufs=1) as cspool, \
         tc.tile_pool(name="w", bufs=1) as wpool, \
         tc.tile_pool(name="xs", bufs=2) as xspool, \
         tc.tile_pool(name="rope", bufs=4) as rpool, \
         tc.tile_pool(name="pp", bufs=1, space="PSUM") as pp:
        cos_sb = cspool.tile([P, S], F32)
        sin_sb = cspool.tile([P, S], F32)
        wq_sb = wpool.tile([P, NJ, F], BF16)
        wk_sb = wpool.tile([P, NJ, F], BF16)
        wv_sb = wpool.tile([P, NJ, F], BF16)

        xs_tiles = {}

        def load_xs(s):
            xs = xspool.tile([P, NJ, 512], BF16, name=f"xs{s}", tag="xs")
            xs_tiles[s] = xs
            return xs

        xs0 = load_xs(0)
        for j in range(NJ):
            # x chunk first (feeds the first matmuls), weights right behind
            nc.sync.dma_start(xs0[:, j, :], xT[j * P:(j + 1) * P, 0:512])
            nc.sync.dma_start(wq_sb[:, j, :], wqT[j * P:(j + 1) * P, :])
            nc.sync.dma_start(wk_sb[:, j, :], wkT[j * P:(j + 1) * P, :])

        def rope(dst, sl, sh, h):
            # dst = dst*cos + rowswap(dst)*sin, in place on the slab
            rt = rpool.tile([P, 512], F32, name=f"rt{sh}_{h}", tag="rt")
            nc.sync.dma_start(rt[0:64, :], dst[64:128, :])
            nc.sync.dma_start(rt[64:128, :], dst[0:64, :])
            nc.vector.tensor_mul(rt[:], rt[:], sin_sb[:, sl])
            nc.vector.tensor_mul(dst.bitcast(F32R), dst, cos_sb[:, sl])
            nc.vector.tensor_add(dst.bitcast(F32R), dst, rt[:])

        for s in range(NQ):
            sl = slice(s * 512, (s + 1) * 512)
            xs = xs_tiles[s]
            for hp in range(2):          # head pairs (0,1) and (2,3)
                pq = pp.tile([P, 2, 512], F32, name=f"pq{s}_{hp}", tag="acc",
                             bufs=4)
                pk = pp.tile([P, 2, 512], F32, name=f"pk{s}_{hp}", tag="acc",
                             bufs=4)
                for j in range(NJ):
                    for t in range(2):
                        h = 2 * hp + t
                        nc.tensor.matmul(pq[:, t, :],
                                         wq_sb[:, j, h * HD:(h + 1) * HD],
                                         xs[:, j, :],
                                         start=(j == 0), stop=(j == NJ - 1))
                        nc.tensor.matmul(pk[:, t, :],
                                         wk_sb[:, j, h * HD:(h + 1) * HD],
                                         xs[:, j, :],
                                         start=(j == 0), stop=(j == NJ - 1))
                    if s == 0 and hp == 0:
                        # spread the remaining input DMA across the first
                        # compute loop: wv, then cos/sin (needed at first
                        # eviction), then the next x chunk
                        nc.sync.dma_start(wv_sb[:, j, :],
                                          wvT[j * P:(j + 1) * P, :])
                        if j == 2:
                            nc.sync.dma_start(cos_sb[:], cosT)
                            nc.sync.dma_start(sin_sb[:], sinT)
                if s == 0 and hp == 1:
                    xs1 = load_xs(1)
                    for j in range(NJ):
                        nc.sync.dma_start(xs1[:, j, :],
                                          xT[j * P:(j + 1) * P, 512:1024])
                # evictions split ScalarE/VectorE; RoPE on VectorE
                qdst = qT[:, 2 * hp:2 * hp + 2, sl]
                kdst = kT[:, 2 * hp:2 * hp + 2, sl]
                nc.scalar.activation(qdst.bitcast(F32R), pq[:], AFT.Copy)
                nc.vector.tensor_copy(kdst.bitcast(F32R), pk[:])
                for t in range(2):
                    h = 2 * hp + t
                    rope(qT[:, h, sl], sl, s, f"q{h}")
                    rope(kT[:, h, sl], sl, s, f"k{h}")
            # v: reuse the same x tiles as stationaries
            pvA = pp.tile([P, 2, 512], F32, name=f"pvA{s}", tag="acc", bufs=4)
            pvB = pp.tile([P, 2, 512], F32, name=f"pvB{s}", tag="acc", bufs=4)
            for j in range(NJ):
                for st in range(2):
                    nc.tensor.matmul(pvA[:, st, :],
                                     xs[:, j, st * P:(st + 1) * P],
                                     wv_sb[:, j, :],
                                     start=(j == 0), stop=(j == NJ - 1))
                    nc.tensor.matmul(pvB[:, st, :],
                                     xs[:, j, (st + 2) * P:(st + 3) * P],
                                     wv_sb[:, j, :],
                                     start=(j == 0), stop=(j == NJ - 1))
                if s < NQ - 1 and s >= 1:
                    xsn = load_xs(s + 1) if j == 0 else xs_tiles[s + 1]
                    nc.sync.dma_start(xsn[:, j, :],
                                      xT[j * P:(j + 1) * P,
                                         (s + 1) * 512:(s + 2) * 512])
            nc.scalar.activation(vN[:, 4 * s:4 * s + 2, :].bitcast(F32R),
                                 pvA[:], AFT.Copy)
            nc.vector.tensor_copy(vN[:, 4 * s + 2:4 * s + 4, :].bitcast(F32R),
                                  pvB[:])


def _attn_phase(tc, mskT, woT, qT, kT, vN, oT, wo_sb):
    """Causal attention, one global software pipeline over (h, q, j-pair)."""
    nc = tc.nc
    with tc.tile_pool(name="amsk", bufs=1, side="right") as mpool, \
         tc.tile_pool(name="exp", bufs=5, side="right") as epool, \
         tc.tile_pool(name="attsb", bufs=2, side="right") as apool, \
         tc.tile_pool(name="pa", bufs=1, space="PSUM") as pap:
        msk_sb = mpool.tile([P, P], F32)
        nc.sync.dma_start(msk_sb[:], mskT)
        # all-ones [128,128] stationary: the denominator matmul then yields
        # the k-sum already broadcast across all 128 partitions of PSUM.
        ones_tmp = mpool.tile([P, P], F32)
        nc.vector.memset(ones_tmp[:], 1.0)
        ones_mat = mpool.tile([P, P], F32)
        nc.vector.tensor_copy(ones_mat[:].bitcast(F32R), ones_tmp[:])
        # prefetch Wo (fp32) during attention compute
        for h in range(H):
            nc.sync.dma_start(wo_sb[:, h, :], woT[h * P:(h + 1) * P, :])

        # ---- flattened list of score/acc steps across all (h, q) ----
        steps = []
        for h in range(H):
            for q in range(NQ):
                jmax = 4 * (q + 1)
                for jj in range(0, jmax, 2):
                    steps.append((h, q, jj, jmax))
        state = {}  # (h,q) -> dict with psum tiles + ex tiles per step

        def score_step(i):
            h, q, jj, jmax = steps[i]
            qh = qT[:, h, :]
            kh = kT[:, h, :]
            ex = epool.tile([P, 2, 512], F32, name=f"ex{h}_{q}_{jj}", tag="ex")
            psc = pap.tile([P, 2, 512], F32, name=f"psc{h}{q}{jj}",
                           tag="psc", bufs=2)
            offs = []
            for t in range(2):
                j = jj + t
                dd = j - 4 * q
                off = dd * P if dd > 0 else 0
                offs.append(off)
                nc.tensor.matmul(psc[:, t, off:512],
                                 _r(kh[:, j * P:(j + 1) * P]),
                                 _r(qh[:, q * 512 + off:(q + 1) * 512]),
                                 start=True, stop=True)
            if offs[0] == offs[1]:
                # same width: one fused exp eviction for the pair
                nc.scalar.activation(ex[:, 0:2, offs[0]:512].bitcast(F32R),
                                     psc[:, 0:2, offs[0]:512],
                                     AFT.Exp, scale=SCALE)
            else:
                for t in range(2):
                    nc.scalar.activation(ex[:, t, offs[t]:512].bitcast(F32R),
                                         psc[:, t, offs[t]:512],
                                         AFT.Exp, scale=SCALE)
            # triangular mask on the diagonal 128-block
            for t in range(2):
                j = jj + t
                dd = j - 4 * q
                if dd >= 0:
                    nc.vector.tensor_mul(
                        ex[:, t, dd * P:(dd + 1) * P].bitcast(F32R),
                        ex[:, t, dd * P:(dd + 1) * P], msk_sb[:])
            state[(h, q, jj)] = ex

        def acc_step(i):
            h, q, jj, jmax = steps[i]
            ex = state.pop((h, q, jj))
            key = (h, q)
            if jj == 0:
                pden = pap.tile([P, 512], F32, name=f"pden{h}{q}",
                                tag="pden", bufs=2)
                pov = pap.tile([P, 512], F32, name=f"pov{h}{q}",
                               tag="pov", bufs=2)
                state[key] = (pden, pov)
            pden, pov = state[key]
            for t in range(2):
                j = jj + t
                dd = j - 4 * q
                off = dd * P if dd > 0 else 0
                st = (j == 0)
                sp = (j == jmax - 1)
                nc.tensor.matmul(pden[:, off:512], _r(ones_mat[:]),
                                 _r(ex[:, t, off:512]), start=st, stop=sp)
                nc.tensor.matmul(pov[:, off:512],
                                 _r(vN[:, j, h * HD:(h + 1) * HD]),
                                 _r(ex[:, t, off:512]), start=st, stop=sp)
            if jj == jmax - 2:
                del state[key]
                rbc = apool.tile([P, 512], F32, name=f"rbc{h}{q}", tag="rbc")
                nc.vector.reciprocal_approx_fast(rbc[:], pden[:])
                nc.vector.tensor_mul(
                    oT[:, h, q * 512:(q + 1) * 512].bitcast(F32R),
                    pov[:], rbc[:])

        # software pipeline: scores two steps ahead of accumulation
        n = len(steps)
        for i in range(n):
            score_step(i)
            if i >= 2:
                acc_step(i - 2)
        acc_step(n - 2)
        acc_step(n - 1)


def _oproj_phase(tc, out, oT, wo_sb):
    """o_proj, weight-stationary; emits TRANSPOSED partial in bf16."""
    nc = tc.nc
    with tc.tile_pool(name="oev", bufs=4) as oevp, \
         tc.tile_pool(name="po", bufs=1, space="PSUM") as pop:
        for dt in range(D // P):
            po = [pop.tile([P, 512], F32, name=f"po{dt}_{qc}", tag=f"po{qc}",
                           bufs=2) for qc in range(NQ)]
            for h in range(H):
                for qc in range(NQ):
                    nc.tensor.matmul(po[qc][:],
                                     _r(wo_sb[:, h, dt * P:(dt + 1) * P]),
                                     _r(oT[:, h, qc * 512:(qc + 1) * 512]),
                                     start=(h == 0), stop=(h == H - 1))
            for qc in range(NQ):
                ot = oevp.tile([P, 512], BF16, name=f"ot{dt}_{qc}", tag="ot")
                if (dt + qc) % 2 == 0:
                    nc.vector.tensor_copy(ot[:], po[qc][:])
                else:
                    nc.scalar.activation(ot[:], po[qc][:], AFT.Copy)
                nc.sync.dma_start(out[dt * P:(dt + 1) * P,
                                      qc * 512:(qc + 1) * 512], ot[:])


def _body(tc, xT, wqT, wkT, wvT, woT, cosT, sinT, mskT, out):
    nc = tc.nc
    # long-lived slabs; left stack for qkv, right for attention-era tensors
    p_qk = tc.alloc_tile_pool(name="p_qk", bufs=1, side="left")
    qT = p_qk.tile([P, H, S], F32)    # [hd, head, seq]
    kT = p_qk.tile([P, H, S], F32)
    p_v = tc.alloc_tile_pool(name="p_v", bufs=1, side="left")
    vN = p_v.tile([P, NJ, F], F32)   # [:, j, :] = v[j*128:(j+1)*128, :]

    _proj_phase(tc, xT, wqT, wkT, wvT, cosT, sinT, qT, kT, vN)

    p_oT = tc.alloc_tile_pool(name="p_oT", bufs=1, side="right")
    oT = p_oT.tile([P, H, S], F32)    # attention output, transposed
    p_wo = tc.alloc_tile_pool(name="p_wo", bufs=1, side="right")
    wo_sb = p_wo.tile([P, H, D], F32R)

    _attn_phase(tc, mskT, woT, qT, kT, vN, oT, wo_sb)

    p_v.release()
    p_qk.release()

    _oproj_phase(tc, out, oT, wo_sb)
    p_wo.release()
    p_oT.release()


def build_nc():
    nc = bacc.Bacc("TRN2", target_bir_lowering=False, debug=False,
                   enable_asserts=True, num_devices=8)
    xT = nc.dram_tensor("xT", [D, S], BF16, kind="ExternalInput").ap()
    wqT = nc.dram_tensor("wqT", [D, F], BF16, kind="ExternalInput").ap()
    wkT = nc.dram_tensor("wkT", [D, F], BF16, kind="ExternalInput").ap()
    wvT = nc.dram_tensor("wvT", [D, F], BF16, kind="ExternalInput").ap()
    woT = nc.dram_tensor("woT", [F, D], F32R, kind="ExternalInput").ap()
    cosT = nc.dram_tensor("cosT", [P, S], F32, kind="ExternalInput").ap()
    sinT = nc.dram_tensor("sinT", [P, S], F32, kind="ExternalInput").ap()
    mskT = nc.dram_tensor("mskT", [P, P], F32, kind="ExternalInput").ap()
    out = nc.dram_tensor("out", [S, D], BF16, kind="ExternalOutput").ap()

    with tile.TileContext(nc) as tc:
        _body(tc, xT, wqT, wkT, wvT, woT, cosT, sinT, mskT, out)
    nc.compile()
    return nc


_CACHE = {}


def _get_nc():
    if "nc" not in _CACHE:
        _CACHE["nc"] = build_nc()
    return _CACHE["nc"]


def _rope_tables():
    hd = HD
    inv = 1.0 / (10000.0 ** (np.arange(0, hd, 2, dtype=np.float32) / np.float32(hd)))
    t = np.arange(S, dtype=np.float32)
    freqs = np.outer(t, inv)                      # [S, 64]
    emb = np.concatenate([freqs, freqs], axis=-1)  # [S, 128]
    cosT = np.cos(emb).T.astype(np.float32).copy()
    sinT = np.sin(emb).T.astype(np.float32).copy()
    sinT[0:64, :] *= -1.0  # sign of rotate_half baked into the table
    return np.ascontiguousarray(cosT), np.ascontiguousarray(sinT)


def _diag_masks():
    kp = np.arange(P)[:, None]
    qf = np.arange(P)[None, :]
    return np.ascontiguousarray((kp <= qf).astype(np.float32))


def _in_maps(x, Wq, Wk, Wv, Wo):
    cosT, sinT = _rope_tables()
    msk = _diag_masks()
    BF = ml_dtypes.bfloat16
    maps = []
    for c in range(8):
        b, g = c // 4, c % 4
        fs = slice(g * F, (g + 1) * F)
        maps.append({
            "xT": np.ascontiguousarray(x[b].T).astype(BF),
            "wqT": np.ascontiguousarray(Wq[fs, :].T).astype(BF),
            "wkT": np.ascontiguousarray(Wk[fs, :].T).astype(BF),
            "wvT": np.ascontiguousarray(Wv[fs, :].T).astype(BF),
            "woT": np.ascontiguousarray(Wo[:, fs].T),
            "cosT": cosT,
            "sinT": sinT,
            "mskT": msk,
        })
    return maps


def run(x, Wq, Wk, Wv, Wo, trace=False, **spmd_kwargs):
    """Run on 8 cores; returns (full_output, BassKernelResults)."""
    x = np.asarray(x, np.float32)
    Wq = np.asarray(Wq, np.float32)
    Wk = np.asarray(Wk, np.float32)
    Wv = np.asarray(Wv, np.float32)
    Wo = np.asarray(Wo, np.float32)
    nc = _get_nc()
    maps = _in_maps(x, Wq, Wk, Wv, Wo)
    res = bass_utils.run_bass_kernel_spmd(nc, maps, core_ids=list(range(8)),
                                          trace=trace, **spmd_kwargs)
    outs = [res.results[c]["out"].astype(np.float32) for c in range(8)]
    full = np.empty((2, S, D), np.float32)
    for b in range(2):
        # each core returns its o_proj partial TRANSPOSED ([D_out, S])
        acc = outs[4 * b] + outs[4 * b + 1] + outs[4 * b + 2] + outs[4 * b + 3]
        full[b] = acc.T
    return full, res


def kernel(x, Wq, Wk, Wv, Wo):
    full, _ = run(x, Wq, Wk, Wv, Wo)
    return full
